# revision 27
# baseline (speedup 1.0000x reference)
"""Causal self-attention kernel for Trainium2 (8 NeuronCores, Bass/Tile).

Problem: B=4, S=2048, D=1024, H=16, HD=64, fp32.
Sharding: core c -> (batch b = c//2, head-group hg = c%2). Each core computes
attention for its batch over 8 heads (features hg*512..hg*512+511 of each of
the k/q/v projection chunks), plus the partial output projection
attn_out_slice @ W_out[rows of this head group].  Host sums the two partial
out-projections per batch and adds nothing else (b_out folded in on hg==0).

Device-side layout choices (no on-device transposes anywhere):
  - host provides x^T [D, S]; K^T/Q^T are produced feature-major [F, S] in
    bf16 by using W as the matmul stationary operand; V is produced
    seq-major [S, 8*65] by using x^T as the stationary operand, with a
    constant-1.0 65th column per head.
  - attention uses the scores-transposed layout S^T[k, q]: QK^T pairs of
    heads run row-tiled (head A in PE rows 0-63, head B in rows 64-127,
    concurrent on HW), exp() on the scalar engine (no max subtraction:
    scores ~ N(0,1)), causal masking as a 0/1 multiply on band tiles only.
  - AV matmuls use M=65 stationaries [v_head | 1]: the 65th output
    partition accumulates the softmax denominator for free (no separate
    ones-matmuls).  1/D is broadcast across the 64 feature rows via a tiny
    DRAM round-trip DMA (0-step partition APs are DRAM-source-only), and
    head B's normalized tile is shifted to partitions 64-127 by an
    SBUF->SBUF DMA (DVE cannot cross lanes).
  - software pipelining: AV trails QK/exp by one k-tile; each q-block's
    normalize/evict chain is deferred into the next q-block's k-loop.
  - single globally-interleaved schedule: all projection matmuls (V, K/Q,
    out) are emitted as small quanta INTO the attention k-tile stream via
    a credit scheduler, so PE slack under the ~1.3us/k-tile exp cadence
    absorbs the projection work instead of running it in ACT-idle phases.
    PSUM (8 banks): scores 2x2, AV accumulator 1x2 (exits via a DVE copy
    whose latency hides behind the exp lag), projections 2x1.
  - DMA issue discipline (dominant non-engine cost): HWDGE engines
    (sync/scalar) issue everything; SWDGE (gpsimd) costs ~10x more
    sequencer time per descriptor. Weight loads are batched (2+1 DMAs),
    the output staging is one bf16 tile per 512-chunk stored with a
    single DRAM-contiguous DMA (host de-interleaves + sums partials in
    fp32), and partition-crossing SBUF DMAs are minimized: only the
    denominator-row reshape (so the reciprocal runs 128 lanes wide, not
    1) and the per-q-block head-B shift remain.
"""

import math
from contextlib import ExitStack

import numpy as np
from ml_dtypes import bfloat16

import concourse.bass as bass
import concourse.tile as tile
from concourse import bacc, mybir
from concourse.bass_utils import run_bass_kernel_spmd

F32 = mybir.dt.float32
BF16 = mybir.dt.bfloat16

def build_nc(S=2048, D=1024, H_pc=8, HD=64, NQ=512, KT=128, reps=1,
             variant="full", debug_sched=False):
    """Build the single-core Bass program (identical program on all cores).

    v2: single globally-interleaved pipeline.  The attention inner loop is
    ACT(exp)-paced (~1.1us per k-tile); all projection matmuls (V, K/Q, out)
    are emitted as small quanta INTO the attention k-tile stream via a credit
    scheduler, so the PE's spare capacity under the exp cadence absorbs the
    projection work instead of running it in ACT-idle phases.

    PSUM (8 banks): scores 2x[128,1024]f32 (4), AV accumulator o single
    [65,1024]f32 (2; exits PSUM via a DVE copy at qi end whose latency hides
    behind the exp lag), projections 2x[128,512]f32 (2).

    reps>1 wraps the whole kernel body in an on-device For_i loop; used only
    for slope-based HW timing (delta wall-time per rep through the axon
    tunnel), never for the graded path.
    """
    F = H_pc * HD          # per-core feature width of each of k/q/v (512)
    HP = F // 128          # head-pairs == 128-wide feature tiles (4)
    DKT = D // 128         # contraction tiles over d_model (8)
    NSEQ = S // NQ         # q blocks (4)
    NST = S // KT          # seq tiles for V (16)
    DM = D // 128          # output d_model tiles (8)
    NCH = S // 512         # 512-wide seq chunks for projections (4)
    BAND = NQ // KT        # k-tiles per q-block on the causal diagonal (4)

    nc = bacc.Bacc("TRN2", target_bir_lowering=False, debug=False, num_devices=8)

    x_t = nc.dram_tensor("x_t", [D, S], BF16, kind="ExternalInput").ap()
    w_k = nc.dram_tensor("w_k", [D, F], BF16, kind="ExternalInput").ap()
    w_q = nc.dram_tensor("w_q", [D, F], BF16, kind="ExternalInput").ap()
    w_v = nc.dram_tensor("w_v", [D, F], BF16, kind="ExternalInput").ap()
    b_k = nc.dram_tensor("b_k", [F, 1], F32, kind="ExternalInput").ap()
    b_q = nc.dram_tensor("b_q", [F, 1], F32, kind="ExternalInput").ap()
    b_v = nc.dram_tensor("b_v", [F], F32, kind="ExternalInput").ap()
    w_o = nc.dram_tensor("w_o", [F, D], BF16, kind="ExternalInput").ap()
    b_o = nc.dram_tensor("b_o", [D, 1], F32, kind="ExternalInput").ap()
    masks = nc.dram_tensor("masks", [128, 2, 128], BF16, kind="ExternalInput").ap()
    if "oldstore" in variant:
        out_t = nc.dram_tensor("out_t", [D, S], F32, kind="ExternalOutput").ap()
    else:
        out_t = nc.dram_tensor("out_t", [NCH, 128, DM * 512], BF16,
                               kind="ExternalOutput").ap()

    scale = 1.0 / math.sqrt(HD)

    with tile.TileContext(nc) as tc, ExitStack() as ctx:
        consts = ctx.enter_context(tc.tile_pool(name="consts", bufs=1))
        # per-partition bias columns for the feature-major K/Q projections
        bk_sb = consts.tile([128, HP], F32, tag="bk")
        bq_sb = consts.tile([128, HP], F32, tag="bq")
        nc.sync.dma_start(out=bk_sb, in_=b_k.rearrange("(m p) one -> p (m one)", p=128))
        nc.sync.dma_start(out=bq_sb, in_=b_q.rearrange("(m p) one -> p (m one)", p=128))
        # V bias broadcast along partitions (bias varies along the free dim)
        bv_sb = consts.tile([128, F], F32, tag="bv")
        bv_bcast = bass.AP(tensor=b_v.tensor, offset=b_v.offset, ap=[[0, 128], [1, F]])
        nc.sync.dma_start(out=bv_sb, in_=bv_bcast)
        bo_sb = consts.tile([128, DM], F32, tag="bo")
        nc.sync.dma_start(out=bo_sb, in_=b_o.rearrange("(m p) one -> p (m one)", p=128))

        # persistent activations.  v stores 65 columns per head: 64 features
        # plus a constant-1 column, so the AV matmul's 65th output partition
        # accumulates the softmax denominator for free.
        HD1 = HD + 1
        FV = H_pc * HD1        # 520
        big = ctx.enter_context(tc.tile_pool(name="big", bufs=1))
        kT = [big.tile([128, S], BF16, tag=f"kT{m}", name=f"kT{m}") for m in range(HP)]
        qT = [big.tile([128, S], BF16, tag=f"qT{m}", name=f"qT{m}") for m in range(HP)]
        v = [big.tile([128, FV], BF16, tag=f"v{st}", name=f"v{st}") for st in range(NST)]
        aT = [big.tile([128, S], BF16, tag=f"aT{m}", name=f"aT{m}") for m in range(HP)]
        # fill v tiles with 1.0 (idle-DVE memsets at startup); the
        # V-projection eviction overwrites the 64 feature columns per head,
        # leaving each head's 65th (denominator) column at 1.0.
        for st in range(NST):
            nc.vector.memset(v[st][:, :], 1.0)

        xp = ctx.enter_context(tc.tile_pool(name="xp", bufs=1))
        mk = ctx.enter_context(tc.tile_pool(name="mk", bufs=1))
        wsp = ctx.enter_context(tc.tile_pool(name="wsp", bufs=1))
        ptp = ctx.enter_context(tc.tile_pool(name="pt_pool", bufs=6))
        rcp = ctx.enter_context(tc.tile_pool(name="rcp", bufs=2))
        # PSUM budget (8 banks): scores 2x2, o 1x2, projections 2x1.
        sp = ctx.enter_context(tc.tile_pool(name="sp", bufs=2, space="PSUM"))
        op = ctx.enter_context(tc.tile_pool(name="op", bufs=1, space="PSUM"))
        pp = ctx.enter_context(tc.tile_pool(name="pp", bufs=2, space="PSUM"))
        wop = ctx.enter_context(tc.tile_pool(name="wop", bufs=1))
        osb = ctx.enter_context(tc.tile_pool(name="os", bufs=2))
        ocp = ctx.enter_context(tc.tile_pool(name="ocp", bufs=2))
        wvp = ctx.enter_context(tc.tile_pool(name="wvp", bufs=1))
        rdp = ctx.enter_context(tc.tile_pool(name="rdp", bufs=2, space="DRAM"))

        def body():
            # one [128, DKT, S] tile for x^T; dma_start issue overhead is
            # ~0.6us of engine-sequencer time each, so batch all loads into
            # few multi-dim-AP DMAs (DRAM side: p stride S, k stride 128*S).
            xt8 = xp.tile([128, DKT * S], BF16, tag="x8", name="x8")
            xt = [xt8[:, k * S:(k + 1) * S] for k in range(DKT)]

            wv8 = wvp.tile([128, DKT * F], BF16, tag="wv8", name="wv8")
            wv = [wv8[:, k * F:(k + 1) * F] for k in range(DKT)]
            # startup-latency-ordered loads: first x slice, V weights, the
            # hp0 K/Q weights, then everything else — so the prologue's
            # V st0-3 and KQ(0,0) matmuls can start a few us in.
            x3 = xt8.rearrange("p (k s) -> p k s", k=DKT)

            def x_load(c0, c1):
                nc.scalar.dma_start(out=x3[:, :, c0:c1], in_=bass.AP(
                    tensor=x_t.tensor, offset=x_t.offset + c0,
                    ap=[[S, 128], [128 * S, DKT], [1, c1 - c0]]))

            x_load(0, 256)
            for h in range(2):
                nc.scalar.dma_start(
                    out=wv8[:, h * 4 * F:(h + 1) * 4 * F], in_=bass.AP(
                        tensor=w_v.tensor, offset=w_v.offset + h * 4 * 128 * F,
                        ap=[[F, 128], [128 * F, 4], [1, F]]))
            # K/Q projection weights: ONE batched DMA each for all 4
            # head-pairs (8 DMAs -> 2: SP HWDGE sequencer time is precious)
            wkq = {}
            if "oldwkq" in variant:
                for php in range(HP):
                    for wi, wdram in ((0, w_k), (1, w_q)):
                        wt1 = wsp.tile([128, DKT * 128], BF16, tag=f"w{php}{wi}",
                                       name="wt1")
                        nc.sync.dma_start(out=wt1, in_=bass.AP(
                            tensor=wdram.tensor, offset=wdram.offset + php * 128,
                            ap=[[F, 128], [128 * F, DKT], [1, 128]]))
                        wkq[(php, wi)] = [wt1[:, k * 128:(k + 1) * 128]
                                          for k in range(DKT)]
            else:
                wk8 = wsp.tile([128, DKT * F], BF16, tag="wk8", name="wk8")
                wq8 = wsp.tile([128, DKT * F], BF16, tag="wq8", name="wq8")
                for wt8, wdram in ((wk8, w_k), (wq8, w_q)):
                    nc.sync.dma_start(out=wt8, in_=bass.AP(
                        tensor=wdram.tensor, offset=wdram.offset,
                        ap=[[F, 128], [128 * F, DKT], [1, F]]))
                for php in range(HP):
                    for wi, wt8 in ((0, wk8), (1, wq8)):
                        wkq[(php, wi)] = [wt8[:, k * F + php * 128:
                                              k * F + (php + 1) * 128]
                                          for k in range(DKT)]
            x_load(256, 512)
            maskt = mk.tile([128, 2, 128], BF16, tag="mask", name="maskt")
            nc.scalar.dma_start(out=maskt, in_=masks)
            for c in range(1, NCH):
                x_load(c * 512, (c + 1) * 512)
            # W_out for this core is 1 MB bf16: preload with one DMA.
            wot8 = wop.tile([128, HP * D], BF16, tag="wot8", name="wot8")
            nc.sync.dma_start(out=wot8, in_=bass.AP(
                tensor=w_o.tensor, offset=w_o.offset,
                ap=[[D, 128], [128 * D, HP], [1, D]]))
            wo = [[wot8[:, k * D + mo * 128:k * D + (mo + 1) * 128]
                   for mo in range(DM)] for k in range(HP)]

            cA = slice(0, 64)
            cB = slice(64, 128)

            if variant == "noproj":
                for t in kT + qT:
                    nc.vector.memset(t[:, :], 0.01)
            if "nofin" in variant:
                for t in aT:
                    nc.vector.memset(t[:, :], 0.01)

            # ---------- projection work as schedulable quanta ----------
            # Each quantum is (key, est_pe_ns, fn).  Quanta are emitted
            # strictly in list order, either by the credit scheduler (during
            # attention k-loops, to fill PE slack under the exp cadence) or
            # by a force-drain when an attention block needs their output.
            quanta = []
            emitted = set()
            sched_log = {'credit': 0, 'forced': 0, 'tail': 0, 'mode': 'credit',
                         'events': []}
            group_open = [False]

            def pop_one():
                key, pe, fn, opens, closes = quanta.pop(0)
                fn()
                if key is not None:
                    emitted.add(key)
                if opens:
                    group_open[0] = True
                if closes:
                    group_open[0] = False
                sched_log[sched_log['mode']] += 1
                return pe

            def pop_to_boundary():
                while quanta and group_open[0]:
                    pop_one()

            def drain_through(key, where=None):
                if key in emitted or not any(q[0] == key for q in quanta):
                    return
                sched_log['mode'] = 'forced'
                n0 = sched_log['forced']
                while key not in emitted:
                    pop_one()
                if debug_sched:
                    sched_log['events'].append(
                        (where, key, sched_log['forced'] - n0))
                sched_log['mode'] = 'credit'

            MM2 = 390.0  # est PE ns for a 2-matmul quantum

            def vproj_group(st, via_queue=True):
                state = {}

                def mk_q(k0, k1, last):
                    def f():
                        if k0 == 0:
                            state['ps'] = pp.tile([128, F], F32, tag="pj", name="pj")
                        ps = state['ps']
                        for k in range(k0, k1):
                            nc.tensor.matmul(
                                ps, xt[k][:, st * 128:(st + 1) * 128], wv[k],
                                start=(k == 0), stop=(k == DKT - 1),
                            )
                        if last:
                            v3 = v[st].rearrange("p (h c) -> p h c", c=HD1)
                            ps3 = ps.rearrange("p (h c) -> p h c", c=HD)
                            bv3 = bv_sb.rearrange("p (h c) -> p h c", c=HD)
                            nc.vector.tensor_add(v3[:, :, 0:HD], ps3, bv3)
                    return f

                for i in range(4):
                    fn = mk_q(2 * i, 2 * i + 2, i == 3)
                    key = ('v', st) if i == 3 else None
                    if via_queue:
                        quanta.append((key, MM2, fn, i == 0, i == 3))
                    else:
                        fn()
                        if key:
                            emitted.add(key)

            def kqproj_group(hp, ch, wi, via_queue=True):
                bias_sb, dstT = ((bk_sb, kT), (bq_sb, qT))[wi]
                state = {}

                def mk_q(k0, k1, last):
                    def f():
                        if k0 == 0:
                            state['ps'] = pp.tile([128, F], F32, tag="pj", name="pj")
                        ps = state['ps']
                        wt = wkq[(hp, wi)]
                        for k in range(k0, k1):
                            nc.tensor.matmul(
                                ps, wt[k], xt[k][:, ch * 512:(ch + 1) * 512],
                                start=(k == 0), stop=(k == DKT - 1),
                            )
                        if last:
                            with nc.allow_low_precision(reason="bf16 k/q"):
                                nc.vector.tensor_scalar_add(
                                    dstT[hp][:, ch * 512:(ch + 1) * 512],
                                    ps, bias_sb[:, hp:hp + 1],
                                )
                    return f

                for i in range(4):
                    fn = mk_q(2 * i, 2 * i + 2, i == 3)
                    key = ('kq', hp, ch, wi) if i == 3 else None
                    if via_queue:
                        quanta.append((key, MM2, fn, i == 0, i == 3))
                    else:
                        fn()
                        if key:
                            emitted.add(key)

            def enqueue_outproj(nch):
                if "oldstore" in variant:
                    for g in range(DM // 4):
                        st8 = {}

                        def mk_mo4(i, g=g, st8=st8):
                            mo = g * 4 + i

                            def f():
                                if i == 0:
                                    st8['og'] = osb.tile([128, 4 * 512], F32,
                                                         tag="otf", name="otf")
                                ps = pp.tile([128, 512], F32, tag="pj",
                                             name="pj")
                                for k in range(HP):
                                    nc.tensor.matmul(
                                        ps, wo[k][mo],
                                        aT[k][:, nch * 512:(nch + 1) * 512],
                                        start=(k == 0), stop=(k == HP - 1),
                                    )
                                nc.vector.tensor_scalar_add(
                                    st8['og'][:, i * 512:(i + 1) * 512], ps,
                                    bo_sb[:, mo:mo + 1])
                            return f

                        def mk_st4(g=g, st8=st8):
                            def f():
                                nc.sync.dma_start(out=bass.AP(
                                    tensor=out_t.tensor,
                                    offset=out_t.offset + g * 4 * 128 * S + nch * 512,
                                    ap=[[S, 128], [128 * S, 4], [1, 512]]),
                                    in_=st8['og'])
                            return f

                        for i in range(4):
                            quanta.append((None, 2 * MM2, mk_mo4(i), True, True))
                        quanta.append((None, 0.0, mk_st4(), False, False))
                    return
                # all 8 mo-blocks share one bf16 SBUF tile and ONE store DMA
                state = {}

                def mk_mo(mo, state=state):
                    def f():
                        if mo == 0:
                            state['og'] = osb.tile([128, DM * 512], BF16,
                                                   tag="ot", name="ot")
                        ps = pp.tile([128, 512], F32, tag="pj", name="pj")
                        for k in range(HP):
                            nc.tensor.matmul(
                                ps, wo[k][mo],
                                aT[k][:, nch * 512:(nch + 1) * 512],
                                start=(k == 0), stop=(k == HP - 1),
                            )
                        with nc.allow_low_precision(reason="bf16 out"):
                            nc.vector.tensor_scalar_add(
                                state['og'][:, mo * 512:(mo + 1) * 512], ps,
                                bo_sb[:, mo:mo + 1])
                    return f

                def mk_store(state=state):
                    def f():
                        nc.sync.dma_start(out=bass.AP(
                            tensor=out_t.tensor,
                            offset=out_t.offset + nch * 128 * DM * 512,
                            ap=[[DM * 512, 128], [1, DM * 512]]),
                            in_=state['og'])
                    return f

                for mo in range(DM):
                    quanta.append((None, 2 * MM2, mk_mo(mo), True, True))
                quanta.append((None, 0.0, mk_store(), False, False))

            # queue order interleaves V and K/Q groups in first-need order
            if variant != "noproj":
                vproj_group(0, via_queue=False)
                kqproj_group(0, 0, 0, via_queue=False)
                kqproj_group(0, 0, 1, via_queue=False)
                for st in (1, 2, 3):
                    vproj_group(st, via_queue=False)
                # first-need order: Q(ch) at qi=ch start, K(ch) at k-tile
                # 4*ch, V(st) at the AV that reads it
                for qi in (1, 2, 3):
                    kqproj_group(0, qi, 1)
                    kqproj_group(0, qi, 0)
                    for st in range(qi * 4, qi * 4 + 4):
                        vproj_group(st)
                for hp in (1, 2, 3):
                    for ch in range(NCH):
                        kqproj_group(hp, ch, 1)
                        kqproj_group(hp, ch, 0)

            credit = [0.0]

            no_pops = [False]

            def pop_credit(budget):
                credit[0] = min(credit[0] + budget, 6000.0)
                while quanta and not no_pops[0] and credit[0] >= quanta[0][1]:
                    credit[0] -= pop_one()

            pending_fin = []   # deferred normalize/evict chains

            if variant == "projonly":
                for t in kT + qT + aT:
                    nc.vector.memset(t[:, :], 0.25)
                while quanta:
                    pop_one()
                for ch in range(NCH):
                    enqueue_outproj(ch)
                while quanta:
                    pop_one()
                return

            # ---------- ACT-paced attention with interleaved projections ----
            for hp in range(HP):
                for qi in range(NSEQ):
                    nkt = (qi + 1) * BAND
                    drain_through(('v', min(NST - 1, nkt - 1)), (hp, qi))
                    drain_through(('kq', hp, qi, 0), (hp, qi))
                    drain_through(('kq', hp, qi, 1), (hp, qi))
                    # o holds head A in bank 0 cols, head B in bank 1 cols;
                    # partition 64 of each accumulates the softmax denominator
                    # (the constant-1 column of v).
                    o = op.tile([128, 2 * NQ], F32, tag="o")
                    qs = slice(qi * NQ, (qi + 1) * NQ)

                    def av_pair(kt, lo, pt):
                        first, last = (kt == 0), (kt == nkt - 1)
                        nc.tensor.matmul(
                            o[0:65, lo:NQ],
                            v[kt][:, hp * 2 * HD1:hp * 2 * HD1 + HD1],
                            pt[:, lo:NQ],
                            start=first, stop=last, tile_position=(0, 0),
                            skip_group_check=True,
                        )
                        nc.tensor.matmul(
                            o[0:65, NQ + lo:2 * NQ],
                            v[kt][:, hp * 2 * HD1 + HD1:hp * 2 * HD1 + 2 * HD1],
                            pt[:, NQ + lo:2 * NQ],
                            start=first, stop=last, tile_position=(0, 0),
                            skip_group_check=True,
                        )

                    prev_av = None  # AV runs one k-tile behind QK/exp so the
                    # in-order PE always has a QK ready while ACT does exp
                    for kt in range(nkt):
                        ks = slice(kt * 128, (kt + 1) * 128)
                        j = kt - (nkt - BAND)
                        # valid q-subrange of this k-tile: q_local >= 128*j
                        lo = 128 * j if j > 0 else 0
                        s2 = sp.tile([128, 2 * NQ], F32, tag="s")
                        qsub = slice(qi * NQ + lo, (qi + 1) * NQ)
                        nc.tensor.matmul(
                            s2[:, lo:NQ], kT[hp][cA, ks], qT[hp][cA, qsub],
                            start=True, stop=True, tile_position=(0, 0),
                        )
                        nc.tensor.matmul(
                            s2[:, NQ + lo:2 * NQ], kT[hp][cB, ks],
                            qT[hp][cB, qsub],
                            start=True, stop=True, tile_position=(64, 0),
                        )
                        pt = ptp.tile([128, 2 * NQ], BF16, tag="p")
                        s2_3 = s2.rearrange("p (h q) -> p h q", h=2)
                        pt_3 = pt.rearrange("p (h q) -> p h q", h=2)
                        if variant == "tinyexp":
                            nc.scalar.activation(
                                pt_3[:, :, lo:lo + 64], s2_3[:, :, lo:lo + 64],
                                mybir.ActivationFunctionType.Exp, scale=scale,
                            )
                        else:
                            nc.scalar.activation(
                                pt_3[:, :, lo:NQ], s2_3[:, :, lo:NQ],
                                mybir.ActivationFunctionType.Exp, scale=scale,
                            )
                        if j >= 0 and "nomask" not in variant:
                            # triangle mask on the first 128 valid columns
                            nc.vector.tensor_mul(
                                pt_3[:, :, lo:lo + 128], pt_3[:, :, lo:lo + 128],
                                maskt,
                            )
                        if prev_av is not None:
                            av_pair(*prev_av)
                        prev_av = (kt, lo, pt)
                        if kt in (1, 3) and pending_fin:
                            # previous q-block's normalize/evict chain, in two
                            # stages so its DMA-latency waits don't block the
                            # in-order DVE queue for long; emitted after this
                            # k-tile's mask so AV's gate is queued first
                            pending_fin.pop(0)()
                        # fill remaining PE slack under this k-tile's exp with
                        # queued projection quanta
                        w = NQ - lo
                        cadence = (2 * w + 466) / 1.2
                        attn_pe = (w / 4.8 + 160) + (2 * w / 4.8 + 120)
                        pop_credit(cadence - attn_pe)
                    av_pair(*prev_av)
                    if "nofin" in variant:
                        if hp == HP - 1 and variant != "noout":
                            enqueue_outproj(qi)
                        continue
                    # exit PSUM promptly (frees o for the next q-block; the
                    # copy's latency hides behind the exp lag since the next
                    # AV waits on its own exp anyway)
                    o_sb = ocp.tile([65, 2 * NQ], F32, tag="osb")
                    nc.vector.tensor_copy(o_sb, o[0:65, :])

                    # 1/denominator (row 64 of each half): reshape to 128
                    # partitions via an SBUF->SBUF DMA so the reciprocal runs
                    # 128-wide, then DRAM-round-trip broadcast over the 64
                    # feature rows, then normalize+evict into aT.
                    shared = {}

                    def fin_a(o_sb=o_sb, shared=shared):
                        # 1/denominator on the whole row (one DVE lane), then
                        # DRAM-round-trip broadcast across the 64 feature
                        # partitions (0-step partition APs are DRAM-source-
                        # only).  HWDGE (sync) issue is cheap; SWDGE (gpsimd)
                        # costs ~10x more sequencer time per descriptor.
                        # reshape the denominator row to [128, 8] (SBUF->SBUF
                        # DMA) so the reciprocal runs 128 lanes wide: a [1, 1024]
                        # single-lane reciprocal blocks the in-order DVE queue
                        # for ~1us and measures ~55us slower end-to-end
                        rs = rcp.tile([128, 2 * NQ // 128], F32, tag="rs",
                                      name="rs")
                        nc.sync.dma_start(out=rs, in_=o_sb[64:65, :])
                        r2v = rcp.tile([128, 2 * NQ // 128], F32,
                                       tag="r2v2", name="r2v2")
                        nc.vector.reciprocal(r2v, rs)
                        rd = rdp.tile([1, 2 * NQ], F32, tag="rd")
                        nc.sync.dma_start(out=rd, in_=r2v)
                        rb = rcp.tile([64, 2 * NQ], F32, tag="rb")
                        nc.sync.dma_start(out=rb, in_=bass.AP(
                            tensor=rd.tensor, offset=rd.offset,
                            ap=[[0, 64], [1, 2 * NQ]]))
                        shared['rb'] = (rb[:, 0:NQ], rb[:, NQ:2 * NQ])

                    def fin_b(hp=hp, qi=qi, o_sb=o_sb, qs=qs, shared=shared):
                        rbA, rbB = shared['rb']
                        # normalize on GPSIMD (all-SBUF operands): keeps the
                        # in-order DVE queue free for the masks that gate AV
                        mul_eng = nc.gpsimd if "gpsmul" in variant else nc.vector
                        mul_eng.tensor_mul(
                            aT[hp][cA, qs], o_sb[0:64, 0:NQ], rbA)
                        stgB = rcp.tile([64, NQ], BF16, tag="stgB")
                        mul_eng.tensor_mul(
                            stgB, o_sb[0:64, NQ:2 * NQ], rbB)
                        # partition shift 0-63 -> 64-127 (DVE can't cross lanes)
                        nc.sync.dma_start(out=aT[hp][cB, qs], in_=stgB)
                        if hp == HP - 1 and variant != "noout":
                            # all four head-pairs of chunk qi evicted: the
                            # out-projection for this 512-chunk becomes runnable
                            enqueue_outproj(qi)
                    pending_fin.append(fin_a)
                    pending_fin.append(fin_b)

            # tail: the last eviction chain's latency hides under the first
            # out-projection quanta (their aT inputs are long since ready)
            while pending_fin:
                pending_fin.pop(0)()
            sched_log['mode'] = 'tail'
            while quanta:
                pop_one()
            if debug_sched:
                print("sched:", {k: sched_log[k] for k in
                                 ('credit', 'forced', 'tail')})
                for ev in sched_log['events']:
                    print("  forced at", ev)

        if reps == 1:
            body()
        else:
            with tc.For_i(0, reps, 1):
                body()

    nc.compile()
    return nc


def make_masks(NQ=512, KT=128):
    # triangle mask for the 128-wide causal boundary, duplicated for 2 heads
    k = np.arange(128)[:, None]
    c = np.arange(128)[None, :]
    keep = (c >= k).astype(np.float32)
    return np.stack([keep, keep], axis=1)  # [128, 2, 128]


def make_in_maps(x, W_in, b_in, W_out, b_out, S, D, H_pc, HD):
    """Build the 8 per-core input maps. Core c -> (batch c//2, head-group c%2)."""
    F = H_pc * HD
    B = x.shape[0]
    n_hg = D // F  # 2
    masks = make_masks()
    in_maps = []
    for c in range(B * n_hg):
        b, hg = c // n_hg, c % n_hg
        cols = slice(hg * F, (hg + 1) * F)
        # W_in chunk order (torch.chunk in the reference): k, q, v
        wk = np.ascontiguousarray(W_in[:, 0 * D:1 * D][:, cols])
        wq = np.ascontiguousarray(W_in[:, 1 * D:2 * D][:, cols])
        wv = np.ascontiguousarray(W_in[:, 2 * D:3 * D][:, cols])
        bk = np.ascontiguousarray(b_in[0 * D:1 * D][cols]).reshape(F, 1)
        bq = np.ascontiguousarray(b_in[1 * D:2 * D][cols]).reshape(F, 1)
        bv = np.ascontiguousarray(b_in[2 * D:3 * D][cols])
        wo = np.ascontiguousarray(W_out[cols, :])
        bo = (b_out if hg == 0 else np.zeros_like(b_out)).reshape(D, 1)
        in_maps.append({
            "x_t": np.ascontiguousarray(x[b].T).astype(bfloat16),
            "w_k": wk.astype(bfloat16), "w_q": wq.astype(bfloat16),
            "w_v": wv.astype(bfloat16),
            "b_k": bk.astype(np.float32), "b_q": bq.astype(np.float32),
            "b_v": bv.astype(np.float32),
            "w_o": wo.astype(bfloat16), "b_o": bo.astype(np.float32),
            "masks": masks.astype(bfloat16),
        })
    return in_maps


_NC_CACHE = {}


def _get_nc(key, **kw):
    if key not in _NC_CACHE:
        _NC_CACHE[key] = build_nc(**kw)
    return _NC_CACHE[key]


def kernel(x, W_in, b_in, W_out, b_out):
    x = np.asarray(x, dtype=np.float32)
    W_in = np.asarray(W_in, dtype=np.float32)
    b_in = np.asarray(b_in, dtype=np.float32)
    W_out = np.asarray(W_out, dtype=np.float32)
    b_out = np.asarray(b_out, dtype=np.float32)

    B, S, D = x.shape          # 4, 2048, 1024
    HD = 64
    H_pc = (D // HD) // 2      # 8 heads per core

    nc = _get_nc((S, D, H_pc), S=S, D=D, H_pc=H_pc, HD=HD)
    in_maps = make_in_maps(x, W_in, b_in, W_out, b_out, S, D, H_pc, HD)
    res = run_bass_kernel_spmd(nc, in_maps, list(range(2 * B)))
    outs = res.results
    out = np.empty((B, S, D), dtype=np.float32)
    DM, NCH = 8, 4
    for b in range(B):
        acc = (outs[2 * b]["out_t"].astype(np.float32)
               + outs[2 * b + 1]["out_t"].astype(np.float32))
        # [nch, p, mo*512+q] -> [d = mo*128+p, s = nch*512+q]
        acc = acc.reshape(NCH, 128, DM, 512)
        out[b] = acc.transpose(2, 1, 0, 3).reshape(D, S).T
    return out


def _pjrt_runner(nc, n_cores):
    """Cached jitted 8-core runner with no donation, for steady-state timing."""
    import jax
    from jax.sharding import Mesh, PartitionSpec, NamedSharding
    from jax.experimental.shard_map import shard_map
    from concourse import bass2jax, mybir as mb
    bass2jax.install_neuronx_cc_hook()

    partition_name = nc.partition_id_tensor.name if nc.partition_id_tensor else None
    in_names, out_names, out_avals, zero_outs = [], [], [], []
    for alloc in nc.m.functions[0].allocations:
        if not isinstance(alloc, mb.MemoryLocationSet):
            continue
        name = alloc.memorylocations[0].name
        if alloc.kind == "ExternalInput":
            if name != partition_name:
                in_names.append(name)
        elif alloc.kind == "ExternalOutput":
            out_names.append(name)
            shape = tuple(alloc.tensor_shape)
            dtype = mb.dt.np(alloc.dtype)
            out_avals.append(jax.core.ShapedArray(shape, dtype))
            zero_outs.append(np.zeros(shape, dtype))
    n_params = len(in_names)
    all_names = in_names + out_names
    if partition_name is not None:
        all_names = all_names + [partition_name]

    def _body(*args):
        operands = list(args)
        if partition_name is not None:
            operands.append(bass2jax.partition_id_tensor())
        outs = bass2jax._bass_exec_p.bind(
            *operands,
            out_avals=tuple(out_avals),
            in_names=tuple(all_names),
            out_names=tuple(out_names),
            lowering_input_output_aliases=(),
            sim_require_finite=True,
            sim_require_nnan=True,
            nc=nc,
        )
        return tuple(outs)

    devices = jax.devices()[:n_cores]
    mesh = Mesh(np.asarray(devices), ("core",))
    spec = PartitionSpec("core")
    f = jax.jit(shard_map(
        _body, mesh=mesh,
        in_specs=(spec,) * (n_params + len(out_names)),
        out_specs=(spec,) * len(out_names),
        check_rep=False,
    ))
    sharding = NamedSharding(mesh, spec)
    return f, in_names, zero_outs, sharding, out_names


def _timed_runner(reps, in_maps):
    """Jitted 8-core runner for the program with an on-device repeat loop."""
    import time as _time
    import jax
    nc = build_nc(reps=reps)
    f, in_names, zero_outs, sharding, out_names = _pjrt_runner(nc, len(in_maps))
    args = []
    for name in in_names:
        g = np.concatenate([np.asarray(in_maps[c][name]) for c in range(len(in_maps))], axis=0)
        args.append(jax.device_put(g, sharding))
    for z in zero_outs:
        g = np.concatenate([z] * len(in_maps), axis=0)
        args.append(jax.device_put(g, sharding))

    def run():
        t0 = _time.perf_counter()
        out = f(*args)
        jax.block_until_ready(out)
        return _time.perf_counter() - t0

    return run


def time_kernel(x, W_in, b_in, W_out, b_out, pairs=8, k2=65):
    """Measure (dispatch_wall_ns, hw_exec_ns).

    A single dispatch through the axon tunnel has a ~67 ms wall-clock floor
    of pure client-server latency (a trivial kernel measures the same), so
    the HW execution time is obtained as the slope of wall time vs on-device
    repeat count: (T(reps=k2) - T(reps=1)) / (k2 - 1), with the reps=1 calls
    interleaved around each reps=k2 call to cancel tunnel drift.
    """
    x = np.asarray(x, dtype=np.float32)
    B, S, D = x.shape
    HD = 64
    H_pc = (D // HD) // 2
    in_maps = make_in_maps(np.asarray(x), np.asarray(W_in), np.asarray(b_in),
                           np.asarray(W_out), np.asarray(b_out), S, D, H_pc, HD)
    r1 = _timed_runner(1, in_maps)
    r2 = _timed_runner(k2, in_maps)
    r1()
    r2()  # warmup
    slopes, walls = [], []
    for _ in range(pairs):
        t1a = r1()
        t2 = r2()
        t1b = r1()
        slopes.append((t2 - (t1a + t1b) / 2) / (k2 - 1) * 1e9)
        walls += [t1a, t1b]
    slopes.sort()
    return min(walls) * 1e9, slopes[len(slopes) // 2]

